# revision 10
# baseline (speedup 1.0000x reference)
"""DeepSeekV2 decoder layer (MLA attention + top-2-of-8 MoE) on 8 TRN2 cores.

Sharding: token-parallel attention (512 tok/core; cores 0-3 = batch 0,
cores 4-7 = batch 1), expert-parallel routed MoE (expert c on core c),
token-parallel shared expert. Collectives: AG1 (kcT+vov, per batch group),
AG2 (xf) + AGL (logits), final ReduceScatter of routed partials.

Self-contained: hardcodes all shapes. kernel(**inputs) -> [B,S,D] fp32.
"""

import sys
import types

import numpy as np

import concourse.bass as bass
import concourse.mybir as mybir
import concourse.tile as tile
from concourse import bacc
from concourse import bass_utils

fp32 = mybir.dt.float32
i32 = mybir.dt.int32
AF = mybir.ActivationFunctionType
ALU = mybir.AluOpType
AX = mybir.AxisListType

B, S, D, H = 2, 2048, 2048, 16
HD, R, RH, LR, FF, E = 128, 64, 32, 64, 2048, 8
T = B * S
NC = 8
TS = T // NC  # 512
P = 128
NT = T // P  # 32
NTS = TS // P  # 4
ND = D // P  # 16
NF = FF // P  # 16
ROPE_BASE = 10000.0
LN_EPS = 1e-5
CAP = 1280
NCT = CAP // P  # 10
HUGE = float(1 << 20)
NPASS = 2
PT = NCT // NPASS  # 5 tiles / pass
PTOK = PT * P  # 640

KC_ROWS = H * LR  # 1024
AG1_KC = KC_ROWS * TS
AG1_SH = AG1_KC + TS * D
GRP = 4


def _install_ntff_shim():
    try:
        import antenv  # noqa

        if "antenv.axon_hooks" in sys.modules:
            return
        mod = types.ModuleType("antenv.axon_hooks")
        _h = []

        def set_axon_ntff_profile_hook(h):
            _h.clear()
            _h.append(h)

        def get_axon_ntff_profile_hook():
            if not _h:
                from trn_agent_boot.trn_boot import _ntff_profile_via_ctypes

                _h.append(_ntff_profile_via_ctypes("/opt/axon/libaxon_pjrt.so"))
            return _h[0]

        mod.set_axon_ntff_profile_hook = set_axon_ntff_profile_hook
        mod.get_axon_ntff_profile_hook = get_axon_ntff_profile_hook
        sys.modules["antenv.axon_hooks"] = mod
        antenv.axon_hooks = mod
    except Exception:
        pass


def _layernorm(nc, pool, out, x, w_bc, b_bc, eps_ap=None):
    mu = pool.tile([P, 1], fp32, tag="ln_mu")
    nc.vector.reduce_sum(mu[:], x[:], axis=AX.X)
    nc.vector.tensor_scalar_mul(mu[:], mu[:], 1.0 / D)
    nc.vector.tensor_scalar(out[:], x[:], mu[:], None, op0=ALU.subtract)
    sq = pool.tile([P, D], fp32, tag="ln_sq")
    var = pool.tile([P, 1], fp32, tag="ln_var")
    nc.scalar.activation(sq[:], out[:], AF.Square, accum_out=var[:])
    std = pool.tile([P, 1], fp32, tag="ln_std")
    nc.scalar.activation(std[:], var[:], AF.Sqrt, bias=eps_ap, scale=1.0 / D)
    rstd = pool.tile([P, 1], fp32, tag="ln_rstd")
    nc.vector.reciprocal(rstd[:], std[:])
    nc.vector.tensor_scalar_mul(out[:], out[:], rstd[:])
    nc.vector.tensor_mul(out[:], out[:], w_bc[:])
    nc.vector.tensor_add(out[:], out[:], b_bc[:])


def build_program(debug=False):
    nc = bacc.Bacc("TRN2", target_bir_lowering=False, debug=False, num_devices=NC)

    def din(name, shape, dtype=fp32):
        return nc.dram_tensor(name, shape, dtype, kind="ExternalInput")

    h_my = din("h_my", [TS, D])
    cosT = din("cosT", [R, TS])
    sinTs = din("sinTs", [R, TS])
    Wq, Wk, Wv, Wo = din("Wq", [D, D]), din("Wk", [D, D]), din("Wv", [D, D]), din("Wo", [D, D])
    Wkc, Wqa = din("Wkc", [HD, LR]), din("Wqa", [HD, LR])
    Wqg, Wvov = din("Wqg", [HD, HD]), din("Wvov", [HD, HD])
    ln1w, ln1b = din("ln1w", [P, D]), din("ln1b", [P, D])
    ln2w, ln2b = din("ln2w", [P, D]), din("ln2b", [P, D])
    Wg = din("Wg", [D, E])
    myexp = din("myexp", [P, E])
    Ws1, Ws3, Ws2 = din("Ws1", [D, FF]), din("Ws3", [D, FF]), din("Ws2", [FF, D])
    We1, We3, We2 = din("We1", [D, FF]), din("We3", [D, FF]), din("We2", [FF, D])
    tri128_in = din("tri128", [P, P])
    id128_in = din("id128", [P, P])
    tris32_in = din("tris32", [NT, NT])
    tvalsf_in = din("tvalsf", [P, NT])

    out_my = nc.dram_tensor("out_my", [TS, D], fp32, kind="ExternalOutput")
    dbg = {}
    if debug:
        for nm, shp in [
            ("dbg_hnew", [TS, D]),
            ("dbg_xf", [TS, D]),
            ("dbg_logits", [T, E]),
            ("dbg_wtid", [CAP, 2]),
            ("dbg_shared", [TS, D]),
            ("dbg_partial", [T, D]),
        ]:
            dbg[nm] = nc.dram_tensor(nm, shp, fp32, kind="ExternalOutput")

    with tile.TileContext(nc) as tc:
        cm_cst = tc.tile_pool(name="cst", bufs=1)
        cst = cm_cst.__enter__()
        cm_dram = tc.tile_pool(name="dram", bufs=1, space="DRAM")
        dram = cm_dram.__enter__()

        tri128 = cst.tile([P, P], fp32)
        nc.sync.dma_start(tri128[:], tri128_in[:])
        id128 = cst.tile([P, P], fp32)
        nc.sync.dma_start(id128[:], id128_in[:])
        tris32 = cst.tile([NT, NT], fp32)
        nc.sync.dma_start(tris32[:], tris32_in[:])
        tvalsf = cst.tile([P, NT], fp32)
        nc.sync.dma_start(tvalsf[:], tvalsf_in[:])
        wkc_sb = cst.tile([HD, LR], fp32)
        nc.sync.dma_start(wkc_sb[:], Wkc[:])
        wqa_sb = cst.tile([HD, LR], fp32)
        nc.sync.dma_start(wqa_sb[:], Wqa[:])
        wqg_sb = cst.tile([HD, HD], fp32)
        nc.sync.dma_start(wqg_sb[:], Wqg[:])
        wvov_sb = cst.tile([HD, HD], fp32)
        nc.sync.dma_start(wvov_sb[:], Wvov[:])
        cos_sb = cst.tile([R, TS], fp32)
        nc.sync.dma_start(cos_sb[:], cosT[:])
        sin_sb = cst.tile([R, TS], fp32)
        nc.sync.dma_start(sin_sb[:], sinTs[:])
        myexp_sb = cst.tile([P, E], fp32)
        nc.sync.dma_start(myexp_sb[:], myexp[:])
        zero_sb = cst.tile([P, 512], fp32)
        nc.vector.memset(zero_sb[:], 0.0)
        eps_sb = cst.tile([P, 1], fp32)
        nc.vector.memset(eps_sb[:], LN_EPS)

        ag1_in = dram.tile([AG1_SH], fp32)
        ag1_out = dram.tile([GRP * AG1_SH], fp32)
        ag2_in = dram.tile([TS, D], fp32)
        ag2_out = dram.tile([T, D], fp32, addr_space="Shared")
        agl_in = dram.tile([TS, E], fp32)
        agl_out = dram.tile([T, E], fp32, addr_space="Shared")
        partial = dram.tile([T, D], fp32)
        rs_out = dram.tile([TS, D], fp32)
        wtid = dram.tile([CAP, 2], fp32)

        partial_zv = partial[:].rearrange("t (dc c) -> (t dc) c", c=512)
        for n in range(4 * T // P):
            nc.sync.dma_start(partial_zv[n * P : (n + 1) * P, :], zero_sb[:])

        # scoped activation pools
        cm_xT = tc.tile_pool(name="p_xT", bufs=1)
        p_xT = cm_xT.__enter__()
        xT = p_xT.tile([P, ND, TS], fp32)
        cm_bd = tc.tile_pool(name="p_bd", bufs=1, side="right")
        p_bd = cm_bd.__enter__()
        qa_all = p_bd.tile([LR, H, TS], fp32)
        qgs_all = p_bd.tile([HD, H, TS], fp32)

        # ===== Stage A: LN1 + transpose =====
        with tc.tile_pool(name="sa", bufs=2) as sa, tc.tile_pool(
            name="saps", bufs=4, space="PSUM"
        ) as saps, tc.tile_pool(name="lnc", bufs=1) as lnc:
            ln1w_sb = lnc.tile([P, D], fp32)
            nc.sync.dma_start(ln1w_sb[:], ln1w[:])
            ln1b_sb = lnc.tile([P, D], fp32)
            nc.sync.dma_start(ln1b_sb[:], ln1b[:])
            for tt in range(NTS):
                ht = sa.tile([P, D], fp32, tag="ht")
                nc.sync.dma_start(ht[:], h_my[tt * P : (tt + 1) * P, :])
                xt = sa.tile([P, D], fp32, tag="xt")
                _layernorm(nc, sa, xt, ht, ln1w_sb, ln1b_sb, eps_sb[:])
                for dt in range(ND):
                    pst = saps.tile([P, P], fp32, tag="tr")
                    nc.tensor.transpose(pst[:], xt[:, dt * P : (dt + 1) * P], id128[:])
                    nc.vector.tensor_copy(xT[:, dt, tt * P : (tt + 1) * P], pst[:])

        # ===== Stage B: per-head QKV + rope + projections =====
        with tc.tile_pool(name="sbw", bufs=2) as sbw, tc.tile_pool(
            name="sbps", bufs=1, space="PSUM"
        ) as sbps, tc.tile_pool(name="sbs", bufs=2) as sbs, tc.tile_pool(
            name="sbps2", bufs=1, space="PSUM"
        ) as sbps2:
            for h in range(H):
                c0 = h * HD
                wq_blk = sbw.tile([P, ND, HD], fp32, tag="wq")
                nc.sync.dma_start(
                    wq_blk[:], Wq[:, c0 : c0 + HD].rearrange("(dt p) c -> p dt c", p=P)
                )
                wk_blk = sbw.tile([P, ND, HD], fp32, tag="wk")
                nc.sync.dma_start(
                    wk_blk[:], Wk[:, c0 : c0 + HD].rearrange("(dt p) c -> p dt c", p=P)
                )
                wv_blk = sbw.tile([P, ND, HD], fp32, tag="wv")
                nc.sync.dma_start(
                    wv_blk[:], Wv[:, c0 : c0 + HD].rearrange("(dt p) c -> p dt c", p=P)
                )
                qp = sbps.tile([HD, TS], fp32, tag="qp")
                kp = sbps.tile([HD, TS], fp32, tag="kp")
                vp = sbps.tile([HD, TS], fp32, tag="vp")
                for dt in range(ND):
                    st, sp = dt == 0, dt == ND - 1
                    nc.tensor.matmul(qp[:], lhsT=wq_blk[:, dt], rhs=xT[:, dt], start=st, stop=sp)
                    nc.tensor.matmul(kp[:], lhsT=wk_blk[:, dt], rhs=xT[:, dt], start=st, stop=sp)
                    nc.tensor.matmul(vp[:], lhsT=wv_blk[:, dt], rhs=xT[:, dt], start=st, stop=sp)
                qs = sbs.tile([HD, TS], fp32, tag="qs")
                nc.vector.tensor_copy(qs[:], qp[:])
                ks = sbs.tile([HD, TS], fp32, tag="ks")
                nc.vector.tensor_copy(ks[:], kp[:])
                vs = sbs.tile([HD, TS], fp32, tag="vs")
                nc.vector.tensor_copy(vs[:], vp[:])
                for ap_ in (qs, ks):
                    rot = sbs.tile([R, TS], fp32, tag="rot")
                    nc.sync.dma_start(rot[:RH, :], ap_[RH:R, :])
                    nc.sync.dma_start(rot[RH:R, :], ap_[:RH, :])
                    t1 = sbs.tile([R, TS], fp32, tag="ropet1")
                    nc.vector.tensor_mul(t1[:], ap_[:R, :], cos_sb[:])
                    nc.vector.tensor_mul(rot[:], rot[:], sin_sb[:])
                    nc.vector.tensor_add(ap_[:R, :], t1[:], rot[:])
                kcp = sbps2.tile([LR, TS], fp32, tag="kcp")
                nc.tensor.matmul(kcp[:], lhsT=wkc_sb[:], rhs=ks[:], start=True, stop=True)
                kc_sb = sbs.tile([LR, TS], fp32, tag="kc")
                nc.vector.tensor_copy(kc_sb[:], kcp[:])
                nc.sync.dma_start(
                    ag1_in[h * LR * TS : (h + 1) * LR * TS].rearrange("(r c) -> r c", c=TS),
                    kc_sb[:],
                )
                qap = sbps2.tile([LR, TS], fp32, tag="qap")
                nc.tensor.matmul(qap[:], lhsT=wqa_sb[:], rhs=qs[:], start=True, stop=True)
                nc.vector.tensor_copy(qa_all[:, h], qap[:])
                qgp = sbps2.tile([HD, TS], fp32, tag="qgp")
                nc.tensor.matmul(qgp[:], lhsT=wqg_sb[:], rhs=qs[:], start=True, stop=True)
                nc.scalar.activation(qgs_all[:, h], qgp[:], AF.Silu)
                vov_reg = ag1_in[AG1_KC:].rearrange("(r c) -> r c", c=D)
                for tt in range(NTS):
                    vvp = sbps2.tile([P, HD], fp32, tag="vvp")
                    nc.tensor.matmul(
                        vvp[:], lhsT=vs[:, tt * P : (tt + 1) * P], rhs=wvov_sb[:],
                        start=True, stop=True,
                    )
                    vv_sb = sbs.tile([P, HD], fp32, tag="vv")
                    nc.vector.tensor_copy(vv_sb[:], vvp[:])
                    nc.sync.dma_start(vov_reg[tt * P : (tt + 1) * P, c0 : c0 + HD], vv_sb[:])

        cm_xT.__exit__(None, None, None)

        nc.gpsimd.collective_compute(
            "AllGather", ALU.bypass,
            replica_groups=[[0, 1, 2, 3], [4, 5, 6, 7]],
            ins=[ag1_in[:]], outs=[ag1_out[:]],
        )

        # ===== Stage D: attention =====
        cm_gat = tc.tile_pool(name="p_gat", bufs=1)
        p_gat = cm_gat.__enter__()
        gat_all = p_gat.tile([HD, H, TS], fp32)
        NKT = GRP * NTS
        with tc.tile_pool(name="sdw", bufs=3) as sdw, tc.tile_pool(
            name="sdps", bufs=2, space="PSUM"
        ) as sdps, tc.tile_pool(name="sdacc", bufs=1, space="PSUM") as sdacc, tc.tile_pool(
            name="sds", bufs=3
        ) as sds:
            for h in range(H):
                up = sdacc.tile([HD, TS], fp32, tag="up")
                lp = sdacc.tile([1, TS], fp32, tag="lp")
                for kt in range(NKT):
                    j, l = kt // NTS, kt % NTS
                    base = j * AG1_SH
                    kc_t = sdw.tile([LR, P], fp32, tag="kct")
                    nc.sync.dma_start(
                        kc_t[:],
                        ag1_out[base + h * LR * TS : base + (h + 1) * LR * TS]
                        .rearrange("(r c) -> r c", c=TS)[:, l * P : (l + 1) * P],
                    )
                    vov_t = sdw.tile([P, HD], fp32, tag="vovt")
                    nc.sync.dma_start(
                        vov_t[:],
                        ag1_out[base + AG1_KC : base + AG1_SH]
                        .rearrange("(r c) -> r c", c=D)[
                            l * P : (l + 1) * P, h * HD : (h + 1) * HD
                        ],
                    )
                    scp = sdps.tile([P, TS], fp32, tag="scp")
                    nc.tensor.matmul(scp[:], lhsT=kc_t[:], rhs=qa_all[:, h], start=True, stop=True)
                    ex = sds.tile([P, TS], fp32, tag="ex")
                    nc.scalar.activation(ex[:], scp[:], AF.Exp, scale=0.125)
                    st, sp = kt == 0, kt == NKT - 1
                    nc.tensor.matmul(up[:], lhsT=vov_t[:], rhs=ex[:], start=st, stop=sp)
                    nc.tensor.matmul(
                        lp[:], lhsT=tri128[:, P - 1 : P], rhs=ex[:], start=st, stop=sp
                    )
                rec = sds.tile([1, TS], fp32, tag="rec")
                nc.vector.reciprocal(rec[:], lp[:])
                bcp = sdps.tile([P, TS], fp32, tag="bcp")
                nc.tensor.matmul(bcp[:], lhsT=tri128[0:1, :], rhs=rec[:], start=True, stop=True)
                nc.vector.tensor_mul(gat_all[:, h], qgs_all[:, h], up[:])
                nc.vector.tensor_mul(gat_all[:, h], gat_all[:, h], bcp[:])

        cm_bd.__exit__(None, None, None)

        # ===== Stage D2: Wo + residual =====
        cm_hnew = tc.tile_pool(name="p_hnew", bufs=1, side="right")
        p_hnew = cm_hnew.__enter__()
        hnew_sb = p_hnew.tile([P, NTS, D], fp32)
        with tc.tile_pool(name="sow", bufs=2) as sow, tc.tile_pool(
            name="sops", bufs=2, space="PSUM"
        ) as sops, tc.tile_pool(name="sos", bufs=2) as sos, tc.tile_pool(
            name="sops2", bufs=4, space="PSUM"
        ) as sops2:
            for dt in range(ND):
                wo_blk = sow.tile([P, ND, P], fp32, tag="wo")
                nc.sync.dma_start(
                    wo_blk[:], Wo[:, dt * P : (dt + 1) * P].rearrange("(k p) c -> p k c", p=P)
                )
                aop = sops.tile([P, TS], fp32, tag="aop")
                for j in range(H):
                    nc.tensor.matmul(
                        aop[:], lhsT=wo_blk[:, j], rhs=gat_all[:, j],
                        start=(j == 0), stop=(j == H - 1),
                    )
                ao = sos.tile([P, TS], fp32, tag="ao")
                nc.vector.tensor_copy(ao[:], aop[:])
                for tt in range(NTS):
                    hres = sos.tile([P, P], fp32, tag="hres")
                    nc.sync.dma_start(
                        hres[:], h_my[tt * P : (tt + 1) * P, dt * P : (dt + 1) * P]
                    )
                    trp = sops2.tile([P, P], fp32, tag="aotr")
                    nc.tensor.transpose(trp[:], ao[:, tt * P : (tt + 1) * P], id128[:])
                    nc.vector.tensor_add(
                        hnew_sb[:, tt, dt * P : (dt + 1) * P], trp[:], hres[:]
                    )

        cm_gat.__exit__(None, None, None)

        if debug:
            for tt in range(NTS):
                nc.sync.dma_start(dbg["dbg_hnew"][tt * P : (tt + 1) * P, :], hnew_sb[:, tt])

        # ===== Stage E: LN2, xfT, logits =====
        cm_xfT = tc.tile_pool(name="p_xfT", bufs=1, side="right")
        p_xfT = cm_xfT.__enter__()
        xfT = p_xfT.tile([P, ND, TS], fp32)
        with tc.tile_pool(name="se", bufs=2) as se, tc.tile_pool(
            name="seps", bufs=4, space="PSUM"
        ) as seps, tc.tile_pool(name="lnc2", bufs=1) as lnc2:
            ln2w_sb = lnc2.tile([P, D], fp32)
            nc.sync.dma_start(ln2w_sb[:], ln2w[:])
            ln2b_sb = lnc2.tile([P, D], fp32)
            nc.sync.dma_start(ln2b_sb[:], ln2b[:])
            wg_blk = lnc2.tile([P, ND, E], fp32)
            nc.sync.dma_start(wg_blk[:], Wg[:].rearrange("(dt p) e -> p dt e", p=P))
            for tt in range(NTS):
                xf = se.tile([P, D], fp32, tag="xf")
                _layernorm(nc, se, xf, hnew_sb[:, tt], ln2w_sb, ln2b_sb, eps_sb[:])
                nc.sync.dma_start(ag2_in[tt * P : (tt + 1) * P, :], xf[:])
                if debug:
                    nc.sync.dma_start(dbg["dbg_xf"][tt * P : (tt + 1) * P, :], xf[:])
                for dt in range(ND):
                    pst = seps.tile([P, P], fp32, tag="tr2")
                    nc.tensor.transpose(pst[:], xf[:, dt * P : (dt + 1) * P], id128[:])
                    nc.vector.tensor_copy(xfT[:, dt, tt * P : (tt + 1) * P], pst[:])
            lgp = seps.tile([E, TS], fp32, tag="lgp", bufs=1)
            for dt in range(ND):
                nc.tensor.matmul(
                    lgp[:], lhsT=wg_blk[:, dt], rhs=xfT[:, dt],
                    start=(dt == 0), stop=(dt == ND - 1),
                )
            lg_sb = se.tile([E, TS], fp32, tag="lgs")
            nc.vector.tensor_copy(lg_sb[:], lgp[:])
            for tt in range(NTS):
                ltp = seps.tile([P, E], fp32, tag="ltr", bufs=1)
                nc.tensor.transpose(ltp[:], lg_sb[:, tt * P : (tt + 1) * P], id128[:E, :E])
                lnat = se.tile([P, E], fp32, tag="lnat")
                nc.vector.tensor_copy(lnat[:], ltp[:])
                nc.sync.dma_start(agl_in[tt * P : (tt + 1) * P, :], lnat[:])

        nc.gpsimd.collective_compute(
            "AllGather", ALU.bypass, replica_groups=[list(range(NC))],
            ins=[agl_in[:]], outs=[agl_out[:]],
        )
        nc.gpsimd.collective_compute(
            "AllGather", ALU.bypass, replica_groups=[list(range(NC))],
            ins=[ag2_in[:]], outs=[ag2_out[:]],
        )

        # ===== Routing (replicated on all cores) =====
        cm_rt = tc.tile_pool(name="p_rt", bufs=1, side="right")
        rt = cm_rt.__enter__()
        with tc.tile_pool(name="rtps", bufs=1, space="PSUM") as rtps:
            lg = rt.tile([P, NT, E], fp32)
            nc.sync.dma_start(lg[:], agl_out[:].rearrange("(n p) e -> p n e", p=P))
            if debug:
                nc.sync.dma_start(dbg["dbg_logits"][:], agl_out[:])
            m1 = rt.tile([P, NT], fp32)
            nc.vector.reduce_max(m1[:], lg[:], axis=AX.X)
            m1b = m1[:].rearrange("p (n e) -> p n e", e=1).to_broadcast([P, NT, E])
            eq = rt.tile([P, NT, E], fp32)
            nc.vector.tensor_tensor(out=eq[:], in0=lg[:], in1=m1b, op=ALU.is_equal)
            l2 = rt.tile([P, NT, E], fp32)
            nc.vector.tensor_scalar(l2[:], eq[:], -1e30, None, op0=ALU.mult)
            nc.vector.tensor_add(l2[:], l2[:], lg[:])
            m2 = rt.tile([P, NT], fp32)
            nc.vector.reduce_max(m2[:], l2[:], axis=AX.X)
            m2b = m2[:].rearrange("p (n e) -> p n e", e=1).to_broadcast([P, NT, E])
            maskge = rt.tile([P, NT, E], fp32)
            nc.vector.tensor_tensor(out=maskge[:], in0=lg[:], in1=m2b, op=ALU.is_ge)
            el = rt.tile([P, NT, E], fp32)
            nc.vector.tensor_tensor(out=el[:], in0=lg[:], in1=m1b, op=ALU.subtract)
            nc.scalar.activation(el[:], el[:], AF.Exp)
            nc.vector.tensor_mul(el[:], el[:], maskge[:])
            ssum = rt.tile([P, NT], fp32)
            nc.vector.reduce_sum(ssum[:], el[:], axis=AX.X)
            rss = rt.tile([P, NT], fp32)
            nc.vector.reciprocal(rss[:], ssum[:])
            rssb = rss[:].rearrange("p (n e) -> p n e", e=1).to_broadcast([P, NT, E])
            nc.vector.tensor_tensor(out=el[:], in0=el[:], in1=rssb, op=ALU.mult)
            myb = myexp_sb[:].rearrange("p (n e) -> p n e", n=1).to_broadcast([P, NT, E])
            nc.vector.tensor_tensor(out=el[:], in0=el[:], in1=myb, op=ALU.mult)
            wmine = rt.tile([P, NT], fp32)
            nc.vector.reduce_sum(wmine[:], el[:], axis=AX.X)
            maskm = rt.tile([P, NT], fp32)
            nc.vector.tensor_scalar(maskm[:], wmine[:], 0.0, None, op0=ALU.is_gt)
            incp = rtps.tile([P, NT], fp32, tag="incp")
            nc.tensor.matmul(incp[:], lhsT=tri128[:], rhs=maskm[:], start=True, stop=True)
            totp = rtps.tile([NT, 1], fp32, tag="totp")
            nc.tensor.matmul(totp[:], lhsT=maskm[:], rhs=tri128[:, P - 1 : P], start=True, stop=True)
            tot_sb = rt.tile([NT, 1], fp32)
            nc.vector.tensor_copy(tot_sb[:], totp[:])
            exclp = rtps.tile([NT, 1], fp32, tag="exclp")
            nc.tensor.matmul(exclp[:], lhsT=tris32[:], rhs=tot_sb[:], start=True, stop=True)
            excl_sb = rt.tile([NT, 1], fp32)
            nc.vector.tensor_copy(excl_sb[:], exclp[:])
            exclrp = rtps.tile([1, NT], fp32, tag="exclrp")
            nc.tensor.transpose(exclrp[:], excl_sb[:], id128[:NT, :NT])
            exclr_sb = rt.tile([1, NT], fp32)
            nc.vector.tensor_copy(exclr_sb[:], exclrp[:])
            bcp2 = rtps.tile([P, NT], fp32, tag="bcp2")
            nc.tensor.matmul(bcp2[:], lhsT=tri128[0:1, :], rhs=exclr_sb[:], start=True, stop=True)
            pos = rt.tile([P, NT], fp32)
            nc.vector.tensor_sub(pos[:], incp[:], maskm[:])
            nc.vector.tensor_add(pos[:], pos[:], bcp2[:])
            slotf = rt.tile([P, NT], fp32)
            nc.vector.tensor_mul(slotf[:], pos[:], maskm[:])
            tmp = rt.tile([P, NT], fp32)
            nc.vector.tensor_scalar(tmp[:], maskm[:], -HUGE, HUGE, op0=ALU.mult, op1=ALU.add)
            nc.vector.tensor_add(slotf[:], slotf[:], tmp[:])
            slot_i = rt.tile([P, NT], i32)
            nc.vector.tensor_copy(slot_i[:], slotf[:])
            hug_sb = rt.tile([P, NCT * 2], fp32)
            nc.vector.memset(hug_sb[:], HUGE)
            nc.sync.dma_start(wtid[:].rearrange("(p k) two -> p (k two)", p=P), hug_sb[:])
            for n in range(NT):
                pack = rt.tile([P, 2], fp32, tag="pack", bufs=4)
                nc.vector.tensor_copy(pack[:, 0:1], tvalsf[:, n : n + 1])
                nc.vector.tensor_copy(pack[:, 1:2], wmine[:, n : n + 1])
                nc.gpsimd.indirect_dma_start(
                    out=wtid[:],
                    out_offset=bass.IndirectOffsetOnAxis(ap=slot_i[:, n : n + 1], axis=0),
                    in_=pack[:],
                    in_offset=None,
                    bounds_check=CAP - 1,
                    oob_is_err=False,
                )
            if debug:
                nc.sync.dma_start(dbg["dbg_wtid"][:], wtid[:])
        cm_rt.__exit__(None, None, None)

        # ===== Shared expert =====
        cm_outp = tc.tile_pool(name="p_outp", bufs=1)
        p_outp = cm_outp.__enter__()
        outp_sb = p_outp.tile([P, NTS, D], fp32)
        with tc.tile_pool(name="shw", bufs=2) as shw, tc.tile_pool(
            name="shps", bufs=1, space="PSUM"
        ) as shps, tc.tile_pool(name="shs", bufs=2) as shs, tc.tile_pool(
            name="shg", bufs=1
        ) as shg, tc.tile_pool(name="sheo", bufs=5, space="PSUM") as sheo:
            gsT = shg.tile([P, NF, TS], fp32)
            for ft in range(NF):
                w1_blk = shw.tile([P, ND, P], fp32, tag="w1")
                nc.sync.dma_start(
                    w1_blk[:], Ws1[:, ft * P : (ft + 1) * P].rearrange("(k p) c -> p k c", p=P)
                )
                w3_blk = shw.tile([P, ND, P], fp32, tag="w3")
                nc.sync.dma_start(
                    w3_blk[:], Ws3[:, ft * P : (ft + 1) * P].rearrange("(k p) c -> p k c", p=P)
                )
                h1p = shps.tile([P, TS], fp32, tag="h1p")
                h3p = shps.tile([P, TS], fp32, tag="h3p")
                for dt in range(ND):
                    st, sp = dt == 0, dt == ND - 1
                    nc.tensor.matmul(h1p[:], lhsT=w1_blk[:, dt], rhs=xfT[:, dt], start=st, stop=sp)
                    nc.tensor.matmul(h3p[:], lhsT=w3_blk[:, dt], rhs=xfT[:, dt], start=st, stop=sp)
                s1 = shs.tile([P, TS], fp32, tag="s1")
                nc.scalar.activation(s1[:], h1p[:], AF.Silu)
                nc.vector.tensor_mul(gsT[:, ft], s1[:], h3p[:])
            for dc in range(4):
                eo = [sheo.tile([P, 512], fp32, tag="eo", name=f"eo_sh_{dc}_{i}") for i in range(NTS)]
                for ft in range(NF):
                    w2_t = shw.tile([P, 512], fp32, tag="w2")
                    nc.sync.dma_start(
                        w2_t[:], Ws2[ft * P : (ft + 1) * P, dc * 512 : (dc + 1) * 512]
                    )
                    for tt in range(NTS):
                        nc.tensor.matmul(
                            eo[tt][:], lhsT=gsT[:, ft, tt * P : (tt + 1) * P], rhs=w2_t[:],
                            start=(ft == 0), stop=(ft == NF - 1),
                        )
                for tt in range(NTS):
                    nc.vector.tensor_add(
                        outp_sb[:, tt, dc * 512 : (dc + 1) * 512],
                        eo[tt][:],
                        hnew_sb[:, tt, dc * 512 : (dc + 1) * 512],
                    )
        cm_xfT.__exit__(None, None, None)
        cm_hnew.__exit__(None, None, None)
        if debug:
            for tt in range(NTS):
                nc.sync.dma_start(dbg["dbg_shared"][tt * P : (tt + 1) * P, :], outp_sb[:, tt])

        # ===== Routed expert (2 passes x 640 tokens) =====
        partial_v = partial[:].rearrange("t (dc c) -> (t dc) c", c=512)
        with tc.tile_pool(name="reg", bufs=1) as reg, tc.tile_pool(
            name="reidx", bufs=1
        ) as reidx, tc.tile_pool(name="rew", bufs=2) as rew, tc.tile_pool(
            name="res", bufs=2
        ) as res:
            for ps_ in range(NPASS):
                xgT = reg.tile([P, ND, PTOK], fp32, tag="xgT")
                gT = reg.tile([P, NF, PTOK], fp32, tag="gT")
                idxs, wts = [], []
                with tc.tile_pool(name=f"retr{ps_}", bufs=4, space="PSUM") as retr:
                    for j in range(PT):
                        ct = ps_ * PT + j
                        wt_t = reidx.tile([P, 2], fp32, tag=f"wt{j}")
                        nc.sync.dma_start(wt_t[:], wtid[ct * P : (ct + 1) * P, :])
                        idx_t = reidx.tile([P, 1], i32, tag=f"idx{j}")
                        nc.vector.tensor_copy(idx_t[:], wt_t[:, 0:1])
                        idxs.append(idx_t)
                        wts.append(wt_t)
                        xg = res.tile([P, D], fp32, tag="xg")
                        nc.gpsimd.indirect_dma_start(
                            out=xg[:], out_offset=None, in_=ag2_out[:],
                            in_offset=bass.IndirectOffsetOnAxis(ap=idx_t[:], axis=0),
                            bounds_check=T - 1, oob_is_err=False,
                        )
                        for dt in range(ND):
                            trp = retr.tile([P, P], fp32, tag="xgtr")
                            nc.tensor.transpose(trp[:], xg[:, dt * P : (dt + 1) * P], id128[:])
                            nc.vector.tensor_copy(xgT[:, dt, j * P : (j + 1) * P], trp[:])
                with tc.tile_pool(name=f"reps{ps_}", bufs=1, space="PSUM") as reps:
                    for ft in range(NF):
                        e1_blk = rew.tile([P, ND, P], fp32, tag="e1")
                        nc.sync.dma_start(
                            e1_blk[:],
                            We1[:, ft * P : (ft + 1) * P].rearrange("(k p) c -> p k c", p=P),
                        )
                        e3_blk = rew.tile([P, ND, P], fp32, tag="e3")
                        nc.sync.dma_start(
                            e3_blk[:],
                            We3[:, ft * P : (ft + 1) * P].rearrange("(k p) c -> p k c", p=P),
                        )
                        h1p = reps.tile([P, PTOK], fp32, tag="h1p")
                        h3p = reps.tile([P, PTOK], fp32, tag="h3p")
                        for dt in range(ND):
                            st, sp = dt == 0, dt == ND - 1
                            for lo, hi in ((0, 512), (512, PTOK)):
                                nc.tensor.matmul(
                                    h1p[:, lo:hi], lhsT=e1_blk[:, dt], rhs=xgT[:, dt, lo:hi],
                                    start=st, stop=sp,
                                )
                                nc.tensor.matmul(
                                    h3p[:, lo:hi], lhsT=e3_blk[:, dt], rhs=xgT[:, dt, lo:hi],
                                    start=st, stop=sp,
                                )
                        s1 = res.tile([P, PTOK], fp32, tag="s1r")
                        nc.scalar.activation(s1[:], h1p[:], AF.Silu)
                        nc.vector.tensor_mul(gT[:, ft], s1[:], h3p[:])
                with tc.tile_pool(name=f"reeo{ps_}", bufs=5, space="PSUM") as reeo:
                    for dc in range(4):
                        eo = [reeo.tile([P, 512], fp32, tag="eor", name=f"eo_re_{ps_}_{dc}_{i}") for i in range(PT)]
                        for ft in range(NF):
                            w2_t = rew.tile([P, 512], fp32, tag="we2")
                            nc.sync.dma_start(
                                w2_t[:], We2[ft * P : (ft + 1) * P, dc * 512 : (dc + 1) * 512]
                            )
                            for j in range(PT):
                                nc.tensor.matmul(
                                    eo[j][:], lhsT=gT[:, ft, j * P : (j + 1) * P], rhs=w2_t[:],
                                    start=(ft == 0), stop=(ft == NF - 1),
                                )
                        for j in range(PT):
                            eow = res.tile([P, 512], fp32, tag="eow")
                            nc.vector.tensor_scalar_mul(eow[:], eo[j][:], wts[j][:, 1:2])
                            idx4 = reidx.tile([P, 1], i32, tag=f"idx4_{j}_{dc}")
                            nc.vector.tensor_scalar(
                                idx4[:], idxs[j][:], 4, dc, op0=ALU.mult, op1=ALU.add
                            )
                            nc.gpsimd.indirect_dma_start(
                                out=partial_v,
                                out_offset=bass.IndirectOffsetOnAxis(ap=idx4[:], axis=0),
                                in_=eow[:],
                                in_offset=None,
                                bounds_check=4 * T - 1,
                                oob_is_err=False,
                            )

        if debug:
            nc.sync.dma_start(dbg["dbg_partial"][:], partial[:])

        # ===== ReduceScatter + final add =====
        nc.gpsimd.collective_compute(
            "ReduceScatter", ALU.add, replica_groups=[list(range(NC))],
            ins=[partial[:]], outs=[rs_out[:]],
        )
        with tc.tile_pool(name="fin", bufs=2) as fin:
            for tt in range(NTS):
                rst = fin.tile([P, D], fp32, tag="rst")
                nc.sync.dma_start(rst[:], rs_out[tt * P : (tt + 1) * P, :])
                ot = fin.tile([P, D], fp32, tag="ot")
                nc.vector.tensor_add(ot[:], rst[:], outp_sb[:, tt])
                nc.sync.dma_start(out_my[tt * P : (tt + 1) * P, :], ot[:])
        cm_outp.__exit__(None, None, None)
        cm_cst.__exit__(None, None, None)
        cm_dram.__exit__(None, None, None)

    nc.compile()
    return nc


def make_in_maps(inputs):
    f32 = lambda x: np.ascontiguousarray(np.asarray(x), dtype=np.float32)
    hs = f32(inputs["hidden_states"]).reshape(T, D)
    pos = np.asarray(inputs["position_ids"]).reshape(-1).astype(np.int64)
    inv_freq = 1.0 / (ROPE_BASE ** (np.arange(0, R, 2, dtype=np.float32) / R))
    tt = np.arange(S, dtype=np.float32)
    freqs = tt[:, None] * inv_freq[None, :]
    emb = np.concatenate([freqs, freqs], -1)
    cos_full = np.cos(emb)[pos].astype(np.float32)
    sin_full = np.sin(emb)[pos].astype(np.float32)
    Wvov = (f32(inputs["Wvc"]) @ f32(inputs["Wov"])).astype(np.float32)
    tri128 = (np.arange(P)[:, None] <= np.arange(P)[None, :]).astype(np.float32)
    id128 = np.eye(P, dtype=np.float32)
    tris32 = (np.arange(NT)[:, None] < np.arange(NT)[None, :]).astype(np.float32)
    tvalsf = (np.arange(NT)[None, :] * P + np.arange(P)[:, None]).astype(np.float32)
    common = dict(
        Wq=f32(inputs["Wq"]), Wk=f32(inputs["Wk"]), Wv=f32(inputs["Wv"]),
        Wo=f32(inputs["Wo"]), Wkc=f32(inputs["Wkc"]), Wqa=f32(inputs["Wqa"]),
        Wqg=f32(inputs["Wqg"]), Wvov=Wvov,
        ln1w=np.ascontiguousarray(np.broadcast_to(f32(inputs["ln1_w"]), (P, D))),
        ln1b=np.ascontiguousarray(np.broadcast_to(f32(inputs["ln1_b"]), (P, D))),
        ln2w=np.ascontiguousarray(np.broadcast_to(f32(inputs["ln2_w"]), (P, D))),
        ln2b=np.ascontiguousarray(np.broadcast_to(f32(inputs["ln2_b"]), (P, D))),
        Wg=f32(inputs["Wg"]), Ws1=f32(inputs["Ws1"]), Ws3=f32(inputs["Ws3"]),
        Ws2=f32(inputs["Ws2"]), tri128=tri128, id128=id128, tris32=tris32,
        tvalsf=tvalsf,
    )
    We1, We3, We2 = f32(inputs["We1"]), f32(inputs["We3"]), f32(inputs["We2"])
    in_maps = []
    for c in range(NC):
        s_lo = (c * TS) % S
        cosT_c = np.ascontiguousarray(cos_full[s_lo : s_lo + TS].T)
        sinT_c = np.ascontiguousarray(sin_full[s_lo : s_lo + TS].T)
        sinTs_c = sinT_c.copy()
        sinTs_c[:RH] *= -1.0
        myexp_c = np.zeros((P, E), np.float32)
        myexp_c[:, c] = 1.0
        m = dict(common)
        m.update(
            h_my=np.ascontiguousarray(hs[c * TS : (c + 1) * TS]),
            cosT=cosT_c, sinTs=sinTs_c, myexp=myexp_c,
            We1=np.ascontiguousarray(We1[c]),
            We3=np.ascontiguousarray(We3[c]),
            We2=np.ascontiguousarray(We2[c]),
        )
        in_maps.append(m)
    return in_maps


_cache = {}


def _get_nc(debug=False):
    key = ("nc", debug)
    if key not in _cache:
        _install_ntff_shim()
        _cache[key] = build_program(debug=debug)
    return _cache[key]


def run(inputs, debug=False, trace=False):
    nc = _get_nc(debug=debug)
    in_maps = make_in_maps(inputs)
    return bass_utils.run_bass_kernel_spmd(
        nc, in_maps, core_ids=list(range(NC)), trace=trace
    )


def kernel(**inputs):
    res = run(inputs, debug=False, trace=False)
    out = np.concatenate([res.results[c]["out_my"] for c in range(NC)], axis=0)
    return out.reshape(B, S, D).astype(np.float32)


# revision 14
# speedup vs baseline: 1.5429x; 1.5429x over previous
"""DeepSeekV2 decoder layer (MLA attention + top-2-of-8 MoE) on 8 TRN2 cores.

Sharding: token-parallel attention (512 tok/core; cores 0-3 = batch 0,
cores 4-7 = batch 1), expert-parallel routed MoE (expert c on core c),
token-parallel shared expert. Collectives: AG1 (kcT+vov, per batch group),
AG2 (xf) + AGL (logits), final ReduceScatter of routed partials.

Self-contained: hardcodes all shapes. kernel(**inputs) -> [B,S,D] fp32.
"""

import sys
import types

import numpy as np

import concourse.bass as bass
import concourse.mybir as mybir
import concourse.tile as tile
from concourse import bacc
from concourse import bass_utils

fp32 = mybir.dt.float32
bf16 = mybir.dt.bfloat16
i32 = mybir.dt.int32
AF = mybir.ActivationFunctionType
ALU = mybir.AluOpType
AX = mybir.AxisListType

B, S, D, H = 2, 2048, 2048, 16
HD, R, RH, LR, FF, E = 128, 64, 32, 64, 2048, 8
T = B * S
NC = 8
TS = T // NC  # 512
P = 128
NT = T // P  # 32
NTS = TS // P  # 4
ND = D // P  # 16
NF = FF // P  # 16
ROPE_BASE = 10000.0
LN_EPS = 1e-5
CAP = 1280
NCT = CAP // P  # 10
HUGE = float(1 << 20)
NPASS = 2
PT = NCT // NPASS  # 5 tiles / pass
PTOK = PT * P  # 640

KC_ROWS = H * LR  # 1024
AG1_KC = KC_ROWS * TS
AG1_SH = AG1_KC + TS * D
GRP = 4


def _install_ntff_shim():
    try:
        import antenv  # noqa

        if "antenv.axon_hooks" in sys.modules:
            return
        mod = types.ModuleType("antenv.axon_hooks")
        _h = []

        def set_axon_ntff_profile_hook(h):
            _h.clear()
            _h.append(h)

        def get_axon_ntff_profile_hook():
            if not _h:
                from trn_agent_boot.trn_boot import _ntff_profile_via_ctypes

                _h.append(_ntff_profile_via_ctypes("/opt/axon/libaxon_pjrt.so"))
            return _h[0]

        mod.set_axon_ntff_profile_hook = set_axon_ntff_profile_hook
        mod.get_axon_ntff_profile_hook = get_axon_ntff_profile_hook
        sys.modules["antenv.axon_hooks"] = mod
        antenv.axon_hooks = mod
    except Exception:
        pass


def _layernorm(nc, pool, out, x, w_bc, b_bc, eps_ap=None):
    mu = pool.tile([P, 1], fp32, tag="ln_mu")
    nc.vector.reduce_sum(mu[:], x[:], axis=AX.X)
    nc.vector.tensor_scalar_mul(mu[:], mu[:], 1.0 / D)
    nc.vector.tensor_scalar(out[:], x[:], mu[:], None, op0=ALU.subtract)
    sq = pool.tile([P, D], fp32, tag="ln_sq")
    var = pool.tile([P, 1], fp32, tag="ln_var")
    nc.scalar.activation(sq[:], out[:], AF.Square, accum_out=var[:])
    std = pool.tile([P, 1], fp32, tag="ln_std")
    nc.scalar.activation(std[:], var[:], AF.Sqrt, bias=eps_ap, scale=1.0 / D)
    rstd = pool.tile([P, 1], fp32, tag="ln_rstd")
    nc.vector.reciprocal(rstd[:], std[:])
    nc.vector.tensor_scalar_mul(out[:], out[:], rstd[:])
    nc.vector.tensor_mul(out[:], out[:], w_bc[:])
    nc.vector.tensor_add(out[:], out[:], b_bc[:])


def build_program(debug=False):
    nc = bacc.Bacc("TRN2", target_bir_lowering=False, debug=False, num_devices=NC)

    def din(name, shape, dtype=fp32):
        return nc.dram_tensor(name, shape, dtype, kind="ExternalInput")

    h_my = din("h_my", [TS, D])
    cosT = din("cosT", [R, TS])
    sinTs = din("sinTs", [R, TS])
    Wq, Wk, Wv, Wo = din("Wq", [D, D]), din("Wk", [D, D]), din("Wv", [D, D]), din("Wo", [D, D])
    Wkc, Wqa = din("Wkc", [HD, LR]), din("Wqa", [HD, LR])
    Wqg, Wvov = din("Wqg", [HD, HD]), din("Wvov", [HD, HD])
    ln1w, ln1b = din("ln1w", [P, D]), din("ln1b", [P, D])
    ln2w, ln2b = din("ln2w", [P, D]), din("ln2b", [P, D])
    Wg = din("Wg", [D, E])
    myexp = din("myexp", [P, E])
    Ws1, Ws3, Ws2 = din("Ws1", [D, FF], bf16), din("Ws3", [D, FF], bf16), din("Ws2", [FF, D], bf16)
    We1, We3, We2 = din("We1", [D, FF], bf16), din("We3", [D, FF], bf16), din("We2", [FF, D], bf16)
    tri128_in = din("tri128", [P, P])
    id128_in = din("id128", [P, P])
    tris32_in = din("tris32", [NT, NT])
    tvalsf_in = din("tvalsf", [P, NT])

    out_my = nc.dram_tensor("out_my", [TS, D], fp32, kind="ExternalOutput")
    dbg = {}
    if debug:
        for nm, shp in [
            ("dbg_hnew", [TS, D]),
            ("dbg_xf", [TS, D]),
            ("dbg_logits", [T, E]),
            ("dbg_wtid", [CAP, 2]),
            ("dbg_shared", [TS, D]),
            ("dbg_partial", [T, D]),
        ]:
            dt_ = bf16 if nm == "dbg_partial" else fp32
            dbg[nm] = nc.dram_tensor(nm, shp, dt_, kind="ExternalOutput")

    with tile.TileContext(nc) as tc:
        cm_cst = tc.tile_pool(name="cst", bufs=1)
        cst = cm_cst.__enter__()
        cm_dram = tc.tile_pool(name="dram", bufs=1, space="DRAM")
        dram = cm_dram.__enter__()

        tri128 = cst.tile([P, P], fp32)
        nc.sync.dma_start(tri128[:], tri128_in[:])
        id128 = cst.tile([P, P], fp32)
        nc.sync.dma_start(id128[:], id128_in[:])
        tris32 = cst.tile([NT, NT], fp32)
        nc.sync.dma_start(tris32[:], tris32_in[:])
        tvalsf = cst.tile([P, NT], fp32)
        nc.sync.dma_start(tvalsf[:], tvalsf_in[:])
        wkc_sb = cst.tile([HD, LR], fp32)
        nc.sync.dma_start(wkc_sb[:], Wkc[:])
        wqa_sb = cst.tile([HD, LR], fp32)
        nc.sync.dma_start(wqa_sb[:], Wqa[:])
        wqg_sb = cst.tile([HD, HD], fp32)
        nc.sync.dma_start(wqg_sb[:], Wqg[:])
        wvov_sb = cst.tile([HD, HD], fp32)
        nc.sync.dma_start(wvov_sb[:], Wvov[:])
        cos_sb = cst.tile([R, TS], fp32)
        nc.sync.dma_start(cos_sb[:], cosT[:])
        sin_sb = cst.tile([R, TS], fp32)
        nc.sync.dma_start(sin_sb[:], sinTs[:])
        myexp_sb = cst.tile([P, E], fp32)
        nc.sync.dma_start(myexp_sb[:], myexp[:])
        zero_sb = cst.tile([P, 512], bf16)
        nc.vector.memset(zero_sb[:], 0.0)
        eps_sb = cst.tile([P, 1], fp32)
        nc.vector.memset(eps_sb[:], LN_EPS)
        id_bf = cst.tile([P, P], bf16)
        nc.vector.tensor_copy(id_bf[:], id128[:])

        ag1_in = dram.tile([AG1_SH], fp32)
        ag1_out = dram.tile([GRP * AG1_SH], fp32)
        ag2_in = dram.tile([TS, D], bf16)
        ag2_out = dram.tile([T, D], bf16, addr_space="Shared")
        agl_in = dram.tile([TS, E], fp32)
        agl_out = dram.tile([T, E], fp32, addr_space="Shared")
        partial = dram.tile([T, D], bf16)
        rs_out = dram.tile([TS, D], bf16)
        wtid = dram.tile([CAP, 2], fp32)

        partial_zv = partial[:].rearrange("t (dc c) -> (t dc) c", c=512)
        for n in range(4 * T // P):
            nc.sync.dma_start(partial_zv[n * P : (n + 1) * P, :], zero_sb[:])

        # scoped activation pools
        cm_xT = tc.tile_pool(name="p_xT", bufs=1)
        p_xT = cm_xT.__enter__()
        xT = p_xT.tile([P, ND, TS], fp32)
        cm_bd = tc.tile_pool(name="p_bd", bufs=1, side="right")
        p_bd = cm_bd.__enter__()
        qa_all = p_bd.tile([LR, H, TS], fp32)
        qgs_all = p_bd.tile([HD, H, TS], fp32)

        # ===== Stage A: LN1 + transpose =====
        with tc.tile_pool(name="sa", bufs=2) as sa, tc.tile_pool(
            name="saps", bufs=4, space="PSUM"
        ) as saps, tc.tile_pool(name="lnc", bufs=1) as lnc:
            ln1w_sb = lnc.tile([P, D], fp32)
            nc.sync.dma_start(ln1w_sb[:], ln1w[:])
            ln1b_sb = lnc.tile([P, D], fp32)
            nc.sync.dma_start(ln1b_sb[:], ln1b[:])
            for tt in range(NTS):
                ht = sa.tile([P, D], fp32, tag="ht")
                nc.sync.dma_start(ht[:], h_my[tt * P : (tt + 1) * P, :])
                xt = sa.tile([P, D], fp32, tag="xt")
                _layernorm(nc, sa, xt, ht, ln1w_sb, ln1b_sb, eps_sb[:])
                for dt in range(ND):
                    pst = saps.tile([P, P], fp32, tag="tr")
                    nc.tensor.transpose(pst[:], xt[:, dt * P : (dt + 1) * P], id128[:])
                    nc.vector.tensor_copy(xT[:, dt, tt * P : (tt + 1) * P], pst[:])

        # ===== Stage B: per-head QKV + rope + projections =====
        with tc.tile_pool(name="sbw", bufs=2) as sbw, tc.tile_pool(
            name="sbps", bufs=1, space="PSUM"
        ) as sbps, tc.tile_pool(name="sbs", bufs=2) as sbs, tc.tile_pool(
            name="sbps2", bufs=1, space="PSUM"
        ) as sbps2:
            for h in range(H):
                c0 = h * HD
                wq_blk = sbw.tile([P, ND, HD], fp32, tag="wq")
                nc.sync.dma_start(
                    wq_blk[:], Wq[:, c0 : c0 + HD].rearrange("(dt p) c -> p dt c", p=P)
                )
                wk_blk = sbw.tile([P, ND, HD], fp32, tag="wk")
                nc.sync.dma_start(
                    wk_blk[:], Wk[:, c0 : c0 + HD].rearrange("(dt p) c -> p dt c", p=P)
                )
                wv_blk = sbw.tile([P, ND, HD], fp32, tag="wv")
                nc.sync.dma_start(
                    wv_blk[:], Wv[:, c0 : c0 + HD].rearrange("(dt p) c -> p dt c", p=P)
                )
                qp = sbps.tile([HD, TS], fp32, tag="qp")
                kp = sbps.tile([HD, TS], fp32, tag="kp")
                vp = sbps.tile([HD, TS], fp32, tag="vp")
                for dt in range(ND):
                    st, sp = dt == 0, dt == ND - 1
                    nc.tensor.matmul(qp[:], lhsT=wq_blk[:, dt], rhs=xT[:, dt], start=st, stop=sp)
                    nc.tensor.matmul(kp[:], lhsT=wk_blk[:, dt], rhs=xT[:, dt], start=st, stop=sp)
                    nc.tensor.matmul(vp[:], lhsT=wv_blk[:, dt], rhs=xT[:, dt], start=st, stop=sp)
                qs = sbs.tile([HD, TS], fp32, tag="qs")
                nc.vector.tensor_copy(qs[:], qp[:])
                ks = sbs.tile([HD, TS], fp32, tag="ks")
                nc.vector.tensor_copy(ks[:], kp[:])
                vs = sbs.tile([HD, TS], fp32, tag="vs")
                nc.vector.tensor_copy(vs[:], vp[:])
                for ap_ in (qs, ks):
                    rot = sbs.tile([R, TS], fp32, tag="rot")
                    nc.sync.dma_start(rot[:RH, :], ap_[RH:R, :])
                    nc.sync.dma_start(rot[RH:R, :], ap_[:RH, :])
                    t1 = sbs.tile([R, TS], fp32, tag="ropet1")
                    nc.vector.tensor_mul(t1[:], ap_[:R, :], cos_sb[:])
                    nc.vector.tensor_mul(rot[:], rot[:], sin_sb[:])
                    nc.vector.tensor_add(ap_[:R, :], t1[:], rot[:])
                kcp = sbps2.tile([LR, TS], fp32, tag="kcp")
                nc.tensor.matmul(kcp[:], lhsT=wkc_sb[:], rhs=ks[:], start=True, stop=True)
                kc_sb = sbs.tile([LR, TS], fp32, tag="kc")
                nc.vector.tensor_copy(kc_sb[:], kcp[:])
                nc.sync.dma_start(
                    ag1_in[h * LR * TS : (h + 1) * LR * TS].rearrange("(r c) -> r c", c=TS),
                    kc_sb[:],
                )
                qap = sbps2.tile([LR, TS], fp32, tag="qap")
                nc.tensor.matmul(qap[:], lhsT=wqa_sb[:], rhs=qs[:], start=True, stop=True)
                nc.vector.tensor_copy(qa_all[:, h], qap[:])
                qgp = sbps2.tile([HD, TS], fp32, tag="qgp")
                nc.tensor.matmul(qgp[:], lhsT=wqg_sb[:], rhs=qs[:], start=True, stop=True)
                nc.scalar.activation(qgs_all[:, h], qgp[:], AF.Silu)
                vov_reg = ag1_in[AG1_KC:].rearrange("(r c) -> r c", c=D)
                for tt in range(NTS):
                    vvp = sbps2.tile([P, HD], fp32, tag="vvp")
                    nc.tensor.matmul(
                        vvp[:], lhsT=vs[:, tt * P : (tt + 1) * P], rhs=wvov_sb[:],
                        start=True, stop=True,
                    )
                    vv_sb = sbs.tile([P, HD], fp32, tag="vv")
                    nc.vector.tensor_copy(vv_sb[:], vvp[:])
                    nc.sync.dma_start(vov_reg[tt * P : (tt + 1) * P, c0 : c0 + HD], vv_sb[:])

        cm_xT.__exit__(None, None, None)

        nc.gpsimd.collective_compute(
            "AllGather", ALU.bypass,
            replica_groups=[[0, 1, 2, 3], [4, 5, 6, 7]],
            ins=[ag1_in[:]], outs=[ag1_out[:]],
        )

        # ===== Stage D: attention =====
        cm_gat = tc.tile_pool(name="p_gat", bufs=1)
        p_gat = cm_gat.__enter__()
        gat_all = p_gat.tile([HD, H, TS], fp32)
        NKT = GRP * NTS
        with tc.tile_pool(name="sdw", bufs=3) as sdw, tc.tile_pool(
            name="sdps", bufs=2, space="PSUM"
        ) as sdps, tc.tile_pool(name="sdacc", bufs=1, space="PSUM") as sdacc, tc.tile_pool(
            name="sds", bufs=3
        ) as sds:
            for h in range(H):
                up = sdacc.tile([HD, TS], fp32, tag="up")
                lp = sdacc.tile([1, TS], fp32, tag="lp")
                for kt in range(NKT):
                    j, l = kt // NTS, kt % NTS
                    base = j * AG1_SH
                    kc_t = sdw.tile([LR, P], fp32, tag="kct")
                    nc.sync.dma_start(
                        kc_t[:],
                        ag1_out[base + h * LR * TS : base + (h + 1) * LR * TS]
                        .rearrange("(r c) -> r c", c=TS)[:, l * P : (l + 1) * P],
                    )
                    vov_t = sdw.tile([P, HD], fp32, tag="vovt")
                    nc.sync.dma_start(
                        vov_t[:],
                        ag1_out[base + AG1_KC : base + AG1_SH]
                        .rearrange("(r c) -> r c", c=D)[
                            l * P : (l + 1) * P, h * HD : (h + 1) * HD
                        ],
                    )
                    scp = sdps.tile([P, TS], fp32, tag="scp")
                    nc.tensor.matmul(scp[:], lhsT=kc_t[:], rhs=qa_all[:, h], start=True, stop=True)
                    ex = sds.tile([P, TS], fp32, tag="ex")
                    nc.scalar.activation(ex[:], scp[:], AF.Exp, scale=0.125)
                    st, sp = kt == 0, kt == NKT - 1
                    nc.tensor.matmul(up[:], lhsT=vov_t[:], rhs=ex[:], start=st, stop=sp)
                    nc.tensor.matmul(
                        lp[:], lhsT=tri128[:, P - 1 : P], rhs=ex[:], start=st, stop=sp
                    )
                rec = sds.tile([1, TS], fp32, tag="rec")
                nc.vector.reciprocal(rec[:], lp[:])
                bcp = sdps.tile([P, TS], fp32, tag="bcp")
                nc.tensor.matmul(bcp[:], lhsT=tri128[0:1, :], rhs=rec[:], start=True, stop=True)
                nc.vector.tensor_mul(gat_all[:, h], qgs_all[:, h], up[:])
                nc.vector.tensor_mul(gat_all[:, h], gat_all[:, h], bcp[:])

        cm_bd.__exit__(None, None, None)

        # ===== Stage D2: Wo + residual =====
        cm_hnew = tc.tile_pool(name="p_hnew", bufs=1, side="right")
        p_hnew = cm_hnew.__enter__()
        hnew_sb = p_hnew.tile([P, NTS, D], fp32)
        with tc.tile_pool(name="sow", bufs=2) as sow, tc.tile_pool(
            name="sops", bufs=2, space="PSUM"
        ) as sops, tc.tile_pool(name="sos", bufs=2) as sos, tc.tile_pool(
            name="sops2", bufs=4, space="PSUM"
        ) as sops2:
            for dt in range(ND):
                wo_blk = sow.tile([P, ND, P], fp32, tag="wo")
                nc.sync.dma_start(
                    wo_blk[:], Wo[:, dt * P : (dt + 1) * P].rearrange("(k p) c -> p k c", p=P)
                )
                aop = sops.tile([P, TS], fp32, tag="aop")
                for j in range(H):
                    nc.tensor.matmul(
                        aop[:], lhsT=wo_blk[:, j], rhs=gat_all[:, j],
                        start=(j == 0), stop=(j == H - 1),
                    )
                ao = sos.tile([P, TS], fp32, tag="ao")
                nc.vector.tensor_copy(ao[:], aop[:])
                for tt in range(NTS):
                    hres = sos.tile([P, P], fp32, tag="hres")
                    nc.sync.dma_start(
                        hres[:], h_my[tt * P : (tt + 1) * P, dt * P : (dt + 1) * P]
                    )
                    trp = sops2.tile([P, P], fp32, tag="aotr")
                    nc.tensor.transpose(trp[:], ao[:, tt * P : (tt + 1) * P], id128[:])
                    nc.vector.tensor_add(
                        hnew_sb[:, tt, dt * P : (dt + 1) * P], trp[:], hres[:]
                    )

        cm_gat.__exit__(None, None, None)

        if debug:
            for tt in range(NTS):
                nc.sync.dma_start(dbg["dbg_hnew"][tt * P : (tt + 1) * P, :], hnew_sb[:, tt])

        # ===== Stage E: LN2, xfT, logits =====
        cm_xfT = tc.tile_pool(name="p_xfT", bufs=1, side="right")
        p_xfT = cm_xfT.__enter__()
        xfT = p_xfT.tile([P, ND, TS], bf16)
        cm_xfT32 = tc.tile_pool(name="p_xfT32", bufs=1, side="right")
        p_xfT32 = cm_xfT32.__enter__()
        xfT32 = p_xfT32.tile([P, ND, TS], fp32)
        with tc.tile_pool(name="se", bufs=2) as se, tc.tile_pool(
            name="seps", bufs=4, space="PSUM"
        ) as seps, tc.tile_pool(name="lnc2", bufs=1) as lnc2:
            ln2w_sb = lnc2.tile([P, D], fp32)
            nc.sync.dma_start(ln2w_sb[:], ln2w[:])
            ln2b_sb = lnc2.tile([P, D], fp32)
            nc.sync.dma_start(ln2b_sb[:], ln2b[:])
            wg_blk = lnc2.tile([P, ND, E], fp32)
            nc.sync.dma_start(wg_blk[:], Wg[:].rearrange("(dt p) e -> p dt e", p=P))
            for tt in range(NTS):
                xf = se.tile([P, D], fp32, tag="xf")
                _layernorm(nc, se, xf, hnew_sb[:, tt], ln2w_sb, ln2b_sb, eps_sb[:])
                xf_bf = se.tile([P, D], bf16, tag="xf_bf")
                nc.vector.tensor_copy(xf_bf[:], xf[:])
                nc.sync.dma_start(ag2_in[tt * P : (tt + 1) * P, :], xf_bf[:])
                if debug:
                    nc.sync.dma_start(dbg["dbg_xf"][tt * P : (tt + 1) * P, :], xf[:])
                for dt in range(ND):
                    pst = seps.tile([P, P], fp32, tag="tr2")
                    nc.tensor.transpose(pst[:], xf[:, dt * P : (dt + 1) * P], id128[:])
                    nc.vector.tensor_copy(xfT[:, dt, tt * P : (tt + 1) * P], pst[:])
                    nc.vector.tensor_copy(xfT32[:, dt, tt * P : (tt + 1) * P], pst[:])
            lgp = seps.tile([E, TS], fp32, tag="lgp", bufs=1)
            for dt in range(ND):
                nc.tensor.matmul(
                    lgp[:], lhsT=wg_blk[:, dt], rhs=xfT32[:, dt],
                    start=(dt == 0), stop=(dt == ND - 1),
                )
            lg_sb = se.tile([E, TS], fp32, tag="lgs")
            nc.vector.tensor_copy(lg_sb[:], lgp[:])
            for tt in range(NTS):
                ltp = seps.tile([P, E], fp32, tag="ltr", bufs=1)
                nc.tensor.transpose(ltp[:], lg_sb[:, tt * P : (tt + 1) * P], id128[:E, :E])
                lnat = se.tile([P, E], fp32, tag="lnat")
                nc.vector.tensor_copy(lnat[:], ltp[:])
                nc.sync.dma_start(agl_in[tt * P : (tt + 1) * P, :], lnat[:])

        cm_xfT32.__exit__(None, None, None)
        nc.gpsimd.collective_compute(
            "AllGather", ALU.bypass, replica_groups=[list(range(NC))],
            ins=[agl_in[:]], outs=[agl_out[:]],
        )
        nc.gpsimd.collective_compute(
            "AllGather", ALU.bypass, replica_groups=[list(range(NC))],
            ins=[ag2_in[:]], outs=[ag2_out[:]],
        )

        # ===== Routing (replicated on all cores) =====
        cm_rt = tc.tile_pool(name="p_rt", bufs=1, side="right")
        rt = cm_rt.__enter__()
        with tc.tile_pool(name="rtps", bufs=1, space="PSUM") as rtps:
            lg = rt.tile([P, NT, E], fp32)
            nc.sync.dma_start(lg[:], agl_out[:].rearrange("(n p) e -> p n e", p=P))
            if debug:
                nc.sync.dma_start(dbg["dbg_logits"][:], agl_out[:])
            m1 = rt.tile([P, NT], fp32)
            nc.vector.reduce_max(m1[:], lg[:], axis=AX.X)
            m1b = m1[:].rearrange("p (n e) -> p n e", e=1).to_broadcast([P, NT, E])
            eq = rt.tile([P, NT, E], fp32)
            nc.vector.tensor_tensor(out=eq[:], in0=lg[:], in1=m1b, op=ALU.is_equal)
            l2 = rt.tile([P, NT, E], fp32)
            nc.vector.tensor_scalar(l2[:], eq[:], -1e30, None, op0=ALU.mult)
            nc.vector.tensor_add(l2[:], l2[:], lg[:])
            m2 = rt.tile([P, NT], fp32)
            nc.vector.reduce_max(m2[:], l2[:], axis=AX.X)
            m2b = m2[:].rearrange("p (n e) -> p n e", e=1).to_broadcast([P, NT, E])
            maskge = rt.tile([P, NT, E], fp32)
            nc.vector.tensor_tensor(out=maskge[:], in0=lg[:], in1=m2b, op=ALU.is_ge)
            el = rt.tile([P, NT, E], fp32)
            nc.vector.tensor_tensor(out=el[:], in0=lg[:], in1=m1b, op=ALU.subtract)
            nc.scalar.activation(el[:], el[:], AF.Exp)
            nc.vector.tensor_mul(el[:], el[:], maskge[:])
            ssum = rt.tile([P, NT], fp32)
            nc.vector.reduce_sum(ssum[:], el[:], axis=AX.X)
            rss = rt.tile([P, NT], fp32)
            nc.vector.reciprocal(rss[:], ssum[:])
            rssb = rss[:].rearrange("p (n e) -> p n e", e=1).to_broadcast([P, NT, E])
            nc.vector.tensor_tensor(out=el[:], in0=el[:], in1=rssb, op=ALU.mult)
            myb = myexp_sb[:].rearrange("p (n e) -> p n e", n=1).to_broadcast([P, NT, E])
            nc.vector.tensor_tensor(out=el[:], in0=el[:], in1=myb, op=ALU.mult)
            wmine = rt.tile([P, NT], fp32)
            nc.vector.reduce_sum(wmine[:], el[:], axis=AX.X)
            maskm = rt.tile([P, NT], fp32)
            nc.vector.tensor_scalar(maskm[:], wmine[:], 0.0, None, op0=ALU.is_gt)
            incp = rtps.tile([P, NT], fp32, tag="incp")
            nc.tensor.matmul(incp[:], lhsT=tri128[:], rhs=maskm[:], start=True, stop=True)
            totp = rtps.tile([NT, 1], fp32, tag="totp")
            nc.tensor.matmul(totp[:], lhsT=maskm[:], rhs=tri128[:, P - 1 : P], start=True, stop=True)
            tot_sb = rt.tile([NT, 1], fp32)
            nc.vector.tensor_copy(tot_sb[:], totp[:])
            exclp = rtps.tile([NT, 1], fp32, tag="exclp")
            nc.tensor.matmul(exclp[:], lhsT=tris32[:], rhs=tot_sb[:], start=True, stop=True)
            excl_sb = rt.tile([NT, 1], fp32)
            nc.vector.tensor_copy(excl_sb[:], exclp[:])
            exclrp = rtps.tile([1, NT], fp32, tag="exclrp")
            nc.tensor.transpose(exclrp[:], excl_sb[:], id128[:NT, :NT])
            exclr_sb = rt.tile([1, NT], fp32)
            nc.vector.tensor_copy(exclr_sb[:], exclrp[:])
            bcp2 = rtps.tile([P, NT], fp32, tag="bcp2")
            nc.tensor.matmul(bcp2[:], lhsT=tri128[0:1, :], rhs=exclr_sb[:], start=True, stop=True)
            pos = rt.tile([P, NT], fp32)
            nc.vector.tensor_sub(pos[:], incp[:], maskm[:])
            nc.vector.tensor_add(pos[:], pos[:], bcp2[:])
            slotf = rt.tile([P, NT], fp32)
            nc.vector.tensor_mul(slotf[:], pos[:], maskm[:])
            tmp = rt.tile([P, NT], fp32)
            nc.vector.tensor_scalar(tmp[:], maskm[:], -HUGE, HUGE, op0=ALU.mult, op1=ALU.add)
            nc.vector.tensor_add(slotf[:], slotf[:], tmp[:])
            slot_i = rt.tile([P, NT], i32)
            nc.vector.tensor_copy(slot_i[:], slotf[:])
            hug_sb = rt.tile([P, NCT * 2], fp32)
            nc.vector.memset(hug_sb[:], HUGE)
            nc.sync.dma_start(wtid[:].rearrange("(p k) two -> p (k two)", p=P), hug_sb[:])
            for n in range(NT):
                pack = rt.tile([P, 2], fp32, tag="pack", bufs=4)
                nc.vector.tensor_copy(pack[:, 0:1], tvalsf[:, n : n + 1])
                nc.vector.tensor_copy(pack[:, 1:2], wmine[:, n : n + 1])
                nc.gpsimd.indirect_dma_start(
                    out=wtid[:],
                    out_offset=bass.IndirectOffsetOnAxis(ap=slot_i[:, n : n + 1], axis=0),
                    in_=pack[:],
                    in_offset=None,
                    bounds_check=CAP - 1,
                    oob_is_err=False,
                )
            if debug:
                nc.sync.dma_start(dbg["dbg_wtid"][:], wtid[:])
        cm_rt.__exit__(None, None, None)

        # ===== Shared expert =====
        cm_outp = tc.tile_pool(name="p_outp", bufs=1)
        p_outp = cm_outp.__enter__()
        outp_sb = p_outp.tile([P, NTS, D], fp32)
        with tc.tile_pool(name="shw", bufs=2) as shw, tc.tile_pool(
            name="shps", bufs=1, space="PSUM"
        ) as shps, tc.tile_pool(name="shs", bufs=2) as shs, tc.tile_pool(
            name="shg", bufs=1
        ) as shg, tc.tile_pool(name="sheo", bufs=5, space="PSUM") as sheo:
            gsT = shg.tile([P, NF, TS], bf16)
            for ft in range(NF):
                w1_blk = shw.tile([P, ND, P], bf16, tag="w1")
                nc.sync.dma_start(
                    w1_blk[:], Ws1[:, ft * P : (ft + 1) * P].rearrange("(k p) c -> p k c", p=P)
                )
                w3_blk = shw.tile([P, ND, P], bf16, tag="w3")
                nc.sync.dma_start(
                    w3_blk[:], Ws3[:, ft * P : (ft + 1) * P].rearrange("(k p) c -> p k c", p=P)
                )
                h1p = shps.tile([P, TS], fp32, tag="h1p")
                h3p = shps.tile([P, TS], fp32, tag="h3p")
                for dt in range(ND):
                    st, sp = dt == 0, dt == ND - 1
                    nc.tensor.matmul(h1p[:], lhsT=w1_blk[:, dt], rhs=xfT[:, dt], start=st, stop=sp)
                    nc.tensor.matmul(h3p[:], lhsT=w3_blk[:, dt], rhs=xfT[:, dt], start=st, stop=sp)
                s1 = shs.tile([P, TS], fp32, tag="s1")
                nc.scalar.activation(s1[:], h1p[:], AF.Silu)
                nc.vector.tensor_mul(gsT[:, ft], s1[:], h3p[:])
            for dc in range(4):
                eo = [sheo.tile([P, 512], fp32, tag="eo", name=f"eo_sh_{dc}_{i}") for i in range(NTS)]
                for ft in range(NF):
                    w2_t = shw.tile([P, 512], bf16, tag="w2")
                    nc.sync.dma_start(
                        w2_t[:], Ws2[ft * P : (ft + 1) * P, dc * 512 : (dc + 1) * 512]
                    )
                    for tt in range(NTS):
                        nc.tensor.matmul(
                            eo[tt][:], lhsT=gsT[:, ft, tt * P : (tt + 1) * P], rhs=w2_t[:],
                            start=(ft == 0), stop=(ft == NF - 1),
                        )
                for tt in range(NTS):
                    nc.vector.tensor_add(
                        outp_sb[:, tt, dc * 512 : (dc + 1) * 512],
                        eo[tt][:],
                        hnew_sb[:, tt, dc * 512 : (dc + 1) * 512],
                    )
        cm_xfT.__exit__(None, None, None)
        cm_hnew.__exit__(None, None, None)
        if debug:
            for tt in range(NTS):
                nc.sync.dma_start(dbg["dbg_shared"][tt * P : (tt + 1) * P, :], outp_sb[:, tt])

        # ===== Routed expert (2 passes x 640 tokens) =====
        partial_v = partial[:].rearrange("t (dc c) -> (t dc) c", c=512)
        with tc.tile_pool(name="reg", bufs=1) as reg, tc.tile_pool(
            name="reidx", bufs=1
        ) as reidx, tc.tile_pool(name="rew", bufs=2) as rew, tc.tile_pool(
            name="res", bufs=2
        ) as res:
            for ps_ in range(NPASS):
                xgT = reg.tile([P, ND, PTOK], bf16, tag="xgT")
                gT = reg.tile([P, NF, PTOK], bf16, tag="gT")
                idxs, wts = [], []
                with tc.tile_pool(name=f"retr{ps_}", bufs=4, space="PSUM") as retr:
                    for j in range(PT):
                        ct = ps_ * PT + j
                        wt_t = reidx.tile([P, 2], fp32, tag=f"wt{j}")
                        nc.sync.dma_start(wt_t[:], wtid[ct * P : (ct + 1) * P, :])
                        idx_t = reidx.tile([P, 1], i32, tag=f"idx{j}")
                        nc.vector.tensor_copy(idx_t[:], wt_t[:, 0:1])
                        idxs.append(idx_t)
                        wts.append(wt_t)
                        xg = res.tile([P, D], bf16, tag="xg")
                        nc.gpsimd.indirect_dma_start(
                            out=xg[:], out_offset=None, in_=ag2_out[:],
                            in_offset=bass.IndirectOffsetOnAxis(ap=idx_t[:], axis=0),
                            bounds_check=T - 1, oob_is_err=False,
                        )
                        for dt in range(ND):
                            trp = retr.tile([P, P], bf16, tag="xgtr")
                            nc.tensor.transpose(trp[:], xg[:, dt * P : (dt + 1) * P], id_bf[:])
                            nc.vector.tensor_copy(xgT[:, dt, j * P : (j + 1) * P], trp[:])
                with tc.tile_pool(name=f"reps{ps_}", bufs=1, space="PSUM") as reps:
                    for ft in range(NF):
                        e1_blk = rew.tile([P, ND, P], bf16, tag="e1")
                        nc.sync.dma_start(
                            e1_blk[:],
                            We1[:, ft * P : (ft + 1) * P].rearrange("(k p) c -> p k c", p=P),
                        )
                        e3_blk = rew.tile([P, ND, P], bf16, tag="e3")
                        nc.sync.dma_start(
                            e3_blk[:],
                            We3[:, ft * P : (ft + 1) * P].rearrange("(k p) c -> p k c", p=P),
                        )
                        h1p = reps.tile([P, PTOK], fp32, tag="h1p")
                        h3p = reps.tile([P, PTOK], fp32, tag="h3p")
                        for dt in range(ND):
                            st, sp = dt == 0, dt == ND - 1
                            for lo, hi in ((0, 512), (512, PTOK)):
                                nc.tensor.matmul(
                                    h1p[:, lo:hi], lhsT=e1_blk[:, dt], rhs=xgT[:, dt, lo:hi],
                                    start=st, stop=sp,
                                )
                                nc.tensor.matmul(
                                    h3p[:, lo:hi], lhsT=e3_blk[:, dt], rhs=xgT[:, dt, lo:hi],
                                    start=st, stop=sp,
                                )
                        s1 = res.tile([P, PTOK], fp32, tag="s1r")
                        nc.scalar.activation(s1[:], h1p[:], AF.Silu)
                        nc.vector.tensor_mul(gT[:, ft], s1[:], h3p[:])
                with tc.tile_pool(name=f"reeo{ps_}", bufs=5, space="PSUM") as reeo:
                    for dc in range(4):
                        eo = [reeo.tile([P, 512], fp32, tag="eor", name=f"eo_re_{ps_}_{dc}_{i}") for i in range(PT)]
                        for ft in range(NF):
                            w2_t = rew.tile([P, 512], bf16, tag="we2")
                            nc.sync.dma_start(
                                w2_t[:], We2[ft * P : (ft + 1) * P, dc * 512 : (dc + 1) * 512]
                            )
                            for j in range(PT):
                                nc.tensor.matmul(
                                    eo[j][:], lhsT=gT[:, ft, j * P : (j + 1) * P], rhs=w2_t[:],
                                    start=(ft == 0), stop=(ft == NF - 1),
                                )
                        for j in range(PT):
                            eow = res.tile([P, 512], bf16, tag="eow")
                            nc.vector.tensor_scalar_mul(eow[:], eo[j][:], wts[j][:, 1:2])
                            idx4 = reidx.tile([P, 1], i32, tag=f"idx4_{j}_{dc}")
                            nc.vector.tensor_scalar(
                                idx4[:], idxs[j][:], 4, dc, op0=ALU.mult, op1=ALU.add
                            )
                            nc.gpsimd.indirect_dma_start(
                                out=partial_v,
                                out_offset=bass.IndirectOffsetOnAxis(ap=idx4[:], axis=0),
                                in_=eow[:],
                                in_offset=None,
                                bounds_check=4 * T - 1,
                                oob_is_err=False,
                            )

        if debug:
            nc.sync.dma_start(dbg["dbg_partial"][:], partial[:])

        # ===== ReduceScatter + final add =====
        nc.gpsimd.collective_compute(
            "ReduceScatter", ALU.add, replica_groups=[list(range(NC))],
            ins=[partial[:]], outs=[rs_out[:]],
        )
        with tc.tile_pool(name="fin", bufs=2) as fin:
            for tt in range(NTS):
                rst = fin.tile([P, D], bf16, tag="rst")
                nc.sync.dma_start(rst[:], rs_out[tt * P : (tt + 1) * P, :])
                ot = fin.tile([P, D], fp32, tag="ot")
                nc.vector.tensor_add(ot[:], rst[:], outp_sb[:, tt])
                nc.sync.dma_start(out_my[tt * P : (tt + 1) * P, :], ot[:])
        cm_outp.__exit__(None, None, None)
        cm_cst.__exit__(None, None, None)
        cm_dram.__exit__(None, None, None)

    nc.compile()
    return nc


def make_in_maps(inputs):
    f32 = lambda x: np.ascontiguousarray(np.asarray(x), dtype=np.float32)
    hs = f32(inputs["hidden_states"]).reshape(T, D)
    pos = np.asarray(inputs["position_ids"]).reshape(-1).astype(np.int64)
    inv_freq = 1.0 / (ROPE_BASE ** (np.arange(0, R, 2, dtype=np.float32) / R))
    tt = np.arange(S, dtype=np.float32)
    freqs = tt[:, None] * inv_freq[None, :]
    emb = np.concatenate([freqs, freqs], -1)
    cos_full = np.cos(emb)[pos].astype(np.float32)
    sin_full = np.sin(emb)[pos].astype(np.float32)
    Wvov = (f32(inputs["Wvc"]) @ f32(inputs["Wov"])).astype(np.float32)
    tri128 = (np.arange(P)[:, None] <= np.arange(P)[None, :]).astype(np.float32)
    id128 = np.eye(P, dtype=np.float32)
    tris32 = (np.arange(NT)[:, None] < np.arange(NT)[None, :]).astype(np.float32)
    tvalsf = (np.arange(NT)[None, :] * P + np.arange(P)[:, None]).astype(np.float32)
    import ml_dtypes

    bfc = lambda x: np.ascontiguousarray(np.asarray(x, dtype=np.float32)).astype(
        ml_dtypes.bfloat16
    )
    common = dict(
        Wq=f32(inputs["Wq"]), Wk=f32(inputs["Wk"]), Wv=f32(inputs["Wv"]),
        Wo=f32(inputs["Wo"]), Wkc=f32(inputs["Wkc"]), Wqa=f32(inputs["Wqa"]),
        Wqg=f32(inputs["Wqg"]), Wvov=Wvov,
        ln1w=np.ascontiguousarray(np.broadcast_to(f32(inputs["ln1_w"]), (P, D))),
        ln1b=np.ascontiguousarray(np.broadcast_to(f32(inputs["ln1_b"]), (P, D))),
        ln2w=np.ascontiguousarray(np.broadcast_to(f32(inputs["ln2_w"]), (P, D))),
        ln2b=np.ascontiguousarray(np.broadcast_to(f32(inputs["ln2_b"]), (P, D))),
        Wg=f32(inputs["Wg"]), Ws1=bfc(inputs["Ws1"]), Ws3=bfc(inputs["Ws3"]),
        Ws2=bfc(inputs["Ws2"]), tri128=tri128, id128=id128, tris32=tris32,
        tvalsf=tvalsf,
    )
    We1, We3, We2 = bfc(inputs["We1"]), bfc(inputs["We3"]), bfc(inputs["We2"])
    in_maps = []
    for c in range(NC):
        s_lo = (c * TS) % S
        cosT_c = np.ascontiguousarray(cos_full[s_lo : s_lo + TS].T)
        sinT_c = np.ascontiguousarray(sin_full[s_lo : s_lo + TS].T)
        sinTs_c = sinT_c.copy()
        sinTs_c[:RH] *= -1.0
        myexp_c = np.zeros((P, E), np.float32)
        myexp_c[:, c] = 1.0
        m = dict(common)
        m.update(
            h_my=np.ascontiguousarray(hs[c * TS : (c + 1) * TS]),
            cosT=cosT_c, sinTs=sinTs_c, myexp=myexp_c,
            We1=np.ascontiguousarray(We1[c]),
            We3=np.ascontiguousarray(We3[c]),
            We2=np.ascontiguousarray(We2[c]),
        )
        in_maps.append(m)
    return in_maps


_cache = {}


def _get_nc(debug=False):
    key = ("nc", debug)
    if key not in _cache:
        _install_ntff_shim()
        _cache[key] = build_program(debug=debug)
    return _cache[key]


def run(inputs, debug=False, trace=False):
    nc = _get_nc(debug=debug)
    in_maps = make_in_maps(inputs)
    return bass_utils.run_bass_kernel_spmd(
        nc, in_maps, core_ids=list(range(NC)), trace=trace
    )


def kernel(**inputs):
    res = run(inputs, debug=False, trace=False)
    out = np.concatenate([res.results[c]["out_my"] for c in range(NC)], axis=0)
    return out.reshape(B, S, D).astype(np.float32)
